# revision 21
# baseline (speedup 1.0000x reference)
"""
Bass/Trainium2 kernel for nn_Attention_72456098284196.

Attention module: QKV projections + partial rotary (first 32 of 64 head
channels, all heads) + softmax attention.  B=2, T=2048, C=1024, H=16, D=64.

Sharding: 8 NeuronCores = 2 batches x 4 head-groups (4 heads each).
Pure tensor/batch parallel -> no collectives; host slices inputs and
concatenates outputs.

Device-side math (per core, all matmuls bf16 with fp32 PSUM accum):
  qT[c,t] = WqT.T @ xqT   (c = 4 heads x 64 ch, two 128-partition c-tiles)
  rotary:  rq = q * cos + (S @ q) * sin, with S the signed splice
           permutation (host constant) and cos/sin built on device from
           the positions input (range-reduced Sin on ScalarE).
  scores^T[s,t] = rkT.T @ rqT per head (K=64, two heads row-packed into
           the 64x128 PE tiling mode)
  expT = Exp(scores^T * 1/sqrt(64)) on ScalarE -> bf16
  outT[d,t] accum over s of [v | 1].T @ expT  (M=65: row 64 accumulates
           the softmax denominator for free)
  out = outT[0:64] * (1/outT[64]) broadcast -> DMA out.
"""

import math
import sys

import numpy as np

if "/opt/trn_rl_repo" not in sys.path:
    sys.path.insert(0, "/opt/trn_rl_repo")

import concourse.bass as bass  # noqa: E402
import concourse.mybir as mybir  # noqa: E402
import concourse.tile as tile  # noqa: E402
from concourse import bacc  # noqa: E402
from concourse.bass_utils import run_bass_kernel_spmd  # noqa: E402

B, T, C = 2, 2048, 1024
NUM_HEADS = 16
HEAD_DIM = 64
N_CORES = 8
HEADS_PER_CORE = NUM_HEADS // (N_CORES // B)  # 4
CO = HEADS_PER_CORE * HEAD_DIM  # 256 out channels per core
N_ROT = 32  # rotated channels per head
MAX_WAVELENGTH = 8192.0

F32 = mybir.dt.float32
BF16 = mybir.dt.bfloat16
I32 = mybir.dt.int32

P = 128  # partitions
TCH = 512  # t chunk (PSUM bank)
KCH = C // P  # 8 contraction chunks
NCT = CO // P  # 2 c-tiles (each = 2 heads x 64)
NST = T // P  # 16 s tiles
NTC = T // TCH  # 4 t chunks
SCALE = 1.0 / math.sqrt(HEAD_DIM)
TWO_PI = 2.0 * math.pi


def _inv_freq_col() -> np.ndarray:
    """[128,1] f32 per-partition inverse frequency for a 2-head c-tile.

    Per 64-ch head slot: d<32 -> rotary inv freq (pairs repeated),
    d>=32 -> 0 (cos(0)=1, sin(0)=0 => passthrough channels survive the
    uniform rotary combine unchanged).
    """
    num_bands = N_ROT // 2  # 16
    freq = MAX_WAVELENGTH ** (
        2.0 / N_ROT * np.linspace(0.0, num_bands, num_bands, dtype=np.float64)
    )
    inv = np.repeat(1.0 / freq, 2)  # [32]
    col = np.zeros((P, 1), np.float32)
    for o in (0, 64):
        col[o : o + N_ROT, 0] = inv
    return col


def _splice_matrix_T() -> np.ndarray:
    """lhsT for the splice matmul, [128,128] bf16 (exact in bf16).

    S maps q -> splice(q) per 2-head c-tile: for rotated pairs
    (2i, 2i+1): out[2i] = -q[2i+1], out[2i+1] = q[2i]; passthrough
    channels -> 0.  Returns S.T so that matmul(out, S.T, q) = S @ q.
    """
    S = np.zeros((P, P), np.float32)
    for o in (0, 64):
        for i in range(N_ROT // 2):
            S[o + 2 * i, o + 2 * i + 1] = -1.0
            S[o + 2 * i + 1, o + 2 * i] = 1.0
    return np.ascontiguousarray(S.T).astype(mybir.dt.np(BF16))


def build_bass() -> bass.Bass:
    nc = bacc.Bacc()

    xq_ext = nc.declare_dram_parameter("xqT", [C, T], F32, isOutput=False)
    xkv_ext = nc.declare_dram_parameter("xkvT", [C, T], F32, isOutput=False)
    wq_ext = nc.declare_dram_parameter("wqT", [C, CO], F32, isOutput=False)
    wk_ext = nc.declare_dram_parameter("wkT", [C, CO], F32, isOutput=False)
    wv_ext = nc.declare_dram_parameter("wvT", [C, CO], F32, isOutput=False)
    bq_ext = nc.declare_dram_parameter("bq", [CO, 1], F32, isOutput=False)
    bk_ext = nc.declare_dram_parameter("bk", [CO, 1], F32, isOutput=False)
    bv_ext = nc.declare_dram_parameter("bv", [CO, 1], F32, isOutput=False)
    qpos_ext = nc.declare_dram_parameter("qpos", [T], I32, isOutput=False)
    kpos_ext = nc.declare_dram_parameter("kpos", [T], I32, isOutput=False)
    sperm_ext = nc.declare_dram_parameter("spermT", [P, P], BF16, isOutput=False)
    invf_ext = nc.declare_dram_parameter("invfreq", [P, 1], F32, isOutput=False)
    out_ext = nc.declare_dram_parameter("out", [CO, T], F32, isOutput=True)

    ExpF = mybir.ActivationFunctionType.Exp
    SinF = mybir.ActivationFunctionType.Sin
    Mul = mybir.AluOpType.mult
    Add = mybir.AluOpType.add

    with tile.TileContext(nc) as tc:
        with (
            tc.tile_pool(name="consts", bufs=1) as consts,
            tc.tile_pool(name="persist", bufs=1) as persist,
        ):
            # ---------------- constants ----------------
            sperm_sb = consts.tile([P, P], BF16, tag="sperm")
            nc.sync.dma_start(out=sperm_sb[:], in_=sperm_ext[:, :])
            invf_sb = consts.tile([P, 1], F32, tag="invf")
            nc.sync.dma_start(out=invf_sb[:], in_=invf_ext[:, :])
            bias_cols = {}
            for name, ext in (("q", bq_ext), ("k", bk_ext)):
                for ct in range(NCT):
                    t_ = consts.tile([P, 1], F32, tag=f"b{name}{ct}")
                    nc.sync.dma_start(out=t_[:], in_=ext[ct * P : (ct + 1) * P, :])
                    bias_cols[(name, ct)] = t_
            bvb_sb = consts.tile([P, CO], F32, tag="bvb")
            nc.sync.dma_start(
                out=bvb_sb[:],
                in_=bass.AP(tensor=bv_ext, offset=0, ap=[[0, P], [1, CO]]),
            )
            halfpi = consts.tile([P, 1], F32, tag="halfpi")
            nc.vector.memset(halfpi[:], math.pi / 2.0)

            rot_sb = {}  # ("q"/"k", ct) -> [128, T] bf16 rotated q/k heads
            for name in ("q", "k"):
                for ct in range(NCT):
                    rot_sb[(name, ct)] = persist.tile(
                        [P, T], BF16, tag=f"r{name}{ct}", name=f"r{name}{ct}"
                    )
            v_sb = [
                persist.tile([P, HEADS_PER_CORE, HEAD_DIM + 1], BF16,
                             tag=f"v{st}", name=f"v{st}")
                for st in range(NST)
            ]

            with tc.tile_pool(name="tables", bufs=1) as tables:
                from contextlib import ExitStack

                proj_stack = ExitStack()
                loads = proj_stack.enter_context(
                    tc.tile_pool(name="loads", bufs=2)
                )
                xw = proj_stack.enter_context(tc.tile_pool(name="xw", bufs=1))
                projtmp = proj_stack.enter_context(
                    tc.tile_pool(name="projtmp", bufs=2)
                )
                psproj = proj_stack.enter_context(
                    tc.tile_pool(name="psproj", bufs=1, space="PSUM")
                )
                psproj2 = proj_stack.enter_context(
                    tc.tile_pool(name="psproj2", bufs=2, space="PSUM")
                )
                rots = proj_stack.enter_context(
                    tc.tile_pool(name="rotscratch", bufs=1)
                )

                # ------------- load + cast weights and x -------------
                # xq + wq first so q-projections start immediately; the
                # k-outer projection loops below consume chunks as they
                # land.  xkv casts split DVE/GpSimd.
                w_bf, x_bf = {}, {}

                def load_w(name, ext, k):
                    wf = loads.tile([P, CO], F32, tag="wf32",
                                    name=f"wf{name}{k}")
                    nc.sync.dma_start(out=wf[:], in_=ext[k * P : (k + 1) * P, :])
                    wb = xw.tile([P, CO], BF16, tag=f"w{name}{k}",
                                 name=f"w{name}{k}")
                    nc.vector.tensor_copy(wb[:], wf[:])
                    w_bf[(name, k)] = wb

                def load_x(name, ext, k, gp):
                    xf = loads.tile([P, T], F32, tag="xf32",
                                    name=f"xf{name}{k}")
                    nc.sync.dma_start(out=xf[:], in_=ext[k * P : (k + 1) * P, :])
                    xb = xw.tile([P, T], BF16, tag=f"x{name}{k}",
                                 name=f"x{name}{k}")
                    (nc.gpsimd if gp else nc.vector).tensor_copy(xb[:], xf[:])
                    x_bf[(name, k)] = xb

                for k in range(KCH):
                    load_x("q", xq_ext, k, gp=(k in (3, 6)))
                    load_w("q", wq_ext, k)
                for k in range(KCH):
                    load_x("kv", xkv_ext, k, gp=(k in (3, 6)))
                    load_w("k", wk_ext, k)
                    load_w("v", wv_ext, k)

                # ------------- rotary cos/sin tables (q first; the k
                # tables reuse the same tiles after the q projections) ----
                cos_t = tables.tile([P, T], F32, tag="cos", name="cos_t")
                sin_t = tables.tile([P, T], F32, tag="sin", name="sin_t")

                def build_tables(ext, tbl_name):
                    pos_i = rots.tile([P, T], I32, tag="rs_a",
                                      name=f"posi{tbl_name}")
                    nc.sync.dma_start(
                        out=pos_i[:],
                        in_=bass.AP(tensor=ext, offset=0, ap=[[0, P], [1, T]]),
                    )
                    pos_f = rots.tile([P, T], F32, tag="rs_b",
                                      name=f"posf{tbl_name}")
                    nc.vector.tensor_copy(pos_f[:], pos_i[:])
                    rad = rots.tile([P, T], F32, tag="rs_c",
                                    name=f"rad{tbl_name}")
                    nc.vector.tensor_scalar_mul(rad[:], pos_f[:], invf_sb[:])
                    # range reduction: x_red = rad - rnd(rad/2pi)*2pi
                    # (the f32->i32 cast rounds to nearest)
                    n_i = rots.tile([P, T], I32, tag="rs_a",
                                    name=f"ni{tbl_name}")
                    nc.vector.tensor_scalar_mul(n_i[:], rad[:], 1.0 / TWO_PI)
                    n_f = rots.tile([P, T], F32, tag="rs_b",
                                    name=f"nf{tbl_name}")
                    nc.vector.tensor_copy(n_f[:], n_i[:])
                    xred = rots.tile([P, T], F32, tag="rs_d",
                                     name=f"xred{tbl_name}")
                    nc.vector.scalar_tensor_tensor(
                        xred[:], n_f[:], -TWO_PI, rad[:], op0=Mul, op1=Add
                    )
                    # Sin LUT is only valid within ~[-pi, pi]:
                    # sin = Sin(xred); cos = Sin(pi/2 - |xred|)
                    axred = rots.tile([P, T], F32, tag="rs_c",
                                      name=f"axred{tbl_name}")
                    nc.scalar.activation(
                        axred[:], xred[:], mybir.ActivationFunctionType.Abs
                    )
                    nc.scalar.activation(sin_t[:], xred[:], SinF)
                    nc.scalar.activation(
                        cos_t[:], axred[:], SinF, bias=halfpi[:], scale=-1.0
                    )

                build_tables(qpos_ext, "q")

                # ------------- q/k projections + rotary -------------
                # k-outer: the 4 t-chunk PSUMs of one c-tile accumulate as
                # x chunks land, so the PE is paced by DMA, not stalled.
                def project_rotate(name, xsrc, ct):
                    dst = rot_sb[(name, ct)]
                    qsb = projtmp.tile([P, T], BF16, tag="qlin",
                                       name=f"qlin{name}{ct}")
                    pss = [
                        psproj.tile([P, TCH], F32, tag=f"pslin{tci}",
                                    name=f"ps{name}{ct}_{tci}")
                        for tci in range(NTC)
                    ]
                    for k in range(KCH):
                        for tci in range(NTC):
                            tsl = slice(tci * TCH, (tci + 1) * TCH)
                            nc.tensor.matmul(
                                pss[tci][:],
                                w_bf[(name, k)][:, ct * P : (ct + 1) * P],
                                x_bf[(xsrc, k)][:, tsl],
                                start=(k == 0),
                                stop=(k == KCH - 1),
                            )
                    for tci in range(NTC):
                        tsl = slice(tci * TCH, (tci + 1) * TCH)
                        # q = Wx + b  (copy+bias+cast, PSUM -> bf16 SBUF)
                        nc.vector.tensor_scalar_add(
                            qsb[:, tsl], pss[tci][:], bias_cols[(name, ct)][:]
                        )
                        # splice(q) via PE
                        ps2 = psproj2.tile([P, TCH], F32, tag="psspl",
                                          name=f"ps2{name}{ct}_{tci}")
                        nc.tensor.matmul(
                            ps2[:], sperm_sb[:], qsb[:, tsl],
                            start=True, stop=True,
                        )
                        # rq = q*cos + splice(q)*sin
                        t1 = projtmp.tile([P, TCH], F32, tag="rot1",
                                          name=f"t1{name}{ct}_{tci}")
                        nc.vector.tensor_mul(t1[:], qsb[:, tsl], cos_t[:, tsl])
                        t2 = projtmp.tile([P, TCH], F32, tag="rot2",
                                          name=f"t2{name}{ct}_{tci}")
                        nc.vector.tensor_mul(t2[:], ps2[:], sin_t[:, tsl])
                        nc.vector.tensor_add(dst[:, tsl], t1[:], t2[:])

                project_rotate("q", "q", 0)
                project_rotate("q", "q", 1)
                build_tables(kpos_ext, "k")
                project_rotate("k", "kv", 0)
                project_rotate("k", "kv", 1)

                # ------------- v projection ([s, c] layout) -------------
                for st in range(NST):
                    vt = v_sb[st]
                    psv = psproj2.tile([P, CO], F32, tag="psv", name=f"psv{st}")
                    for k in range(KCH):
                        nc.tensor.matmul(
                            psv[:],
                            x_bf[("kv", k)][:, st * P : (st + 1) * P],
                            w_bf[("v", k)][:],
                            start=(k == 0),
                            stop=(k == KCH - 1),
                        )
                    nc.vector.tensor_add(
                        vt[:, :, 0:HEAD_DIM],
                        psv[:].rearrange("p (h d) -> p h d", h=HEADS_PER_CORE),
                        bvb_sb[:].rearrange("p (h d) -> p h d", h=HEADS_PER_CORE),
                    )
                    nc.vector.memset(vt[:, :, HEAD_DIM : HEAD_DIM + 1], 1.0)

                proj_stack.close()

            # ---------------- attention ----------------
            # Per (pair, t-half): block 1 = row-packed K=64 scores (64x128
            # PE tiling mode) + exp to SBUF; block 2 = V matmuls (full
            # 128-row mode).  Keeping the two PE tiling modes in separate
            # blocks avoids a mode-switch drain per s-tile, so the paired
            # score matmuls actually run concurrently; the V block of one
            # iteration overlaps the exp of the next on ScalarE.
            TH = 1024  # t half width
            with (
                tc.tile_pool(name="pssc", bufs=2, space="PSUM") as pssc,
                tc.tile_pool(name="psvacc", bufs=2, space="PSUM") as psvacc,
                tc.tile_pool(name="expp", bufs=48) as expp,
                tc.tile_pool(name="outp", bufs=2) as outp,
                tc.tile_pool(name="small", bufs=2) as smallp,
            ):
                for it, (pair, th) in enumerate(
                    [(p_, t_) for p_ in range(NCT) for t_ in range(2)]
                ):
                    rk = rot_sb[("k", pair)]
                    rq = rot_sb[("q", pair)]
                    # -------- block 1: scores + exp --------
                    etiles = []
                    for st in range(NST):
                        ssl = slice(st * P, (st + 1) * P)
                        psA = pssc.tile([P, TH], F32, tag="scps",
                                        name=f"scA{it}_{st}")
                        psB = pssc.tile([P, TH], F32, tag="scps",
                                        name=f"scB{it}_{st}")
                        for rows, pst in ((slice(0, 64), psA),
                                          (slice(64, P), psB)):
                            tp = (rows.start, 0)
                            for tcc in range(2):
                                tsl = slice(th * TH + tcc * TCH,
                                            th * TH + (tcc + 1) * TCH)
                                psl = slice(tcc * TCH, (tcc + 1) * TCH)
                                nc.tensor.matmul(
                                    pst[:, psl], rk[rows, ssl],
                                    rq[rows, tsl],
                                    start=True, stop=True,
                                    tile_position=tp,
                                )
                        eA = expp.tile([P, TH], BF16, tag="exp",
                                       name=f"eA{it}_{st}")
                        eB = expp.tile([P, TH], BF16, tag="exp",
                                       name=f"eB{it}_{st}")
                        nc.scalar.activation(eA[:], psA[:], ExpF, scale=SCALE)
                        nc.scalar.activation(eB[:], psB[:], ExpF, scale=SCALE)
                        etiles.append((eA, eB))
                    # -------- block 2: V accumulation --------
                    vps = [
                        psvacc.tile([HEAD_DIM + 1, TH], F32, tag="vacc",
                                    name=f"vacc{it}_{s}")
                        for s in range(2)
                    ]
                    for st in range(NST):
                        for sub in range(2):
                            h = pair * 2 + sub
                            e = etiles[st][sub]
                            for tcc in range(2):
                                psl = slice(tcc * TCH, (tcc + 1) * TCH)
                                nc.tensor.matmul(
                                    vps[sub][:, psl],
                                    v_sb[st][:, h, :],
                                    e[:, psl],
                                    start=(st == 0),
                                    stop=(st == NST - 1),
                                )
                    # -------- epilogue --------
                    vcp = [
                        outp.tile([HEAD_DIM + 1, TH], F32, tag="vcp",
                                  name=f"vcp{it}_{s}")
                        for s in range(2)
                    ]
                    for sub in range(2):
                        nc.vector.tensor_copy(vcp[sub][:], vps[sub][:])
                    dn = smallp.tile([4, TCH], F32, tag="dn", name=f"dn{it}")
                    for sub in range(2):
                        nc.sync.dma_start(
                            out=dn[2 * sub : 2 * sub + 2, :],
                            in_=vcp[sub][HEAD_DIM : HEAD_DIM + 1, :],
                        )
                    rc4 = smallp.tile([4, TCH], F32, tag="rc4", name=f"rc4{it}")
                    nc.vector.reciprocal(rc4[:], dn[:])
                    for sub in range(2):
                        h = pair * 2 + sub
                        rcb = smallp.tile([HEAD_DIM, TH], F32, tag="rcb",
                                          name=f"rcb{it}_{sub}")
                        for j in range(2):
                            r1 = smallp.tile([1, TCH], F32, tag="r1",
                                             name=f"r1_{it}_{sub}{j}")
                            nc.sync.dma_start(
                                out=r1[:],
                                in_=rc4[2 * sub + j : 2 * sub + j + 1, :],
                            )
                            nc.gpsimd.partition_broadcast(
                                rcb[:, j * TCH : (j + 1) * TCH], r1[:],
                                channels=HEAD_DIM,
                            )
                        osb = outp.tile([HEAD_DIM, TH], F32, tag="osb",
                                        name=f"osb{it}_{sub}")
                        nc.vector.tensor_mul(
                            osb[:], vcp[sub][0:HEAD_DIM, :], rcb[:]
                        )
                        nc.sync.dma_start(
                            out=out_ext[h * HEAD_DIM : (h + 1) * HEAD_DIM,
                                        th * TH : (th + 1) * TH],
                            in_=osb[:],
                        )
    nc.finalize()
    return nc


_CACHED = {}


def kernel(x_q, x_kv, q_positions, kv_positions, Wq, bq, Wk, bk, Wv, bv):
    x_q = np.asarray(x_q, np.float32)
    x_kv = np.asarray(x_kv, np.float32)
    q_positions = np.asarray(q_positions, np.int32)
    kv_positions = np.asarray(kv_positions, np.int32)
    Wq, Wk, Wv = (np.asarray(w, np.float32) for w in (Wq, Wk, Wv))
    bq, bk, bv = (np.asarray(b, np.float32) for b in (bq, bk, bv))

    sperm = _splice_matrix_T()
    invf = _inv_freq_col()

    in_maps = []
    for core in range(N_CORES):
        b, hg = divmod(core, N_CORES // B)
        hsl = slice(hg * CO, (hg + 1) * CO)
        in_maps.append(
            {
                "xqT": np.ascontiguousarray(x_q[b].T),
                "xkvT": np.ascontiguousarray(x_kv[b].T),
                "wqT": np.ascontiguousarray(Wq[hsl].T),
                "wkT": np.ascontiguousarray(Wk[hsl].T),
                "wvT": np.ascontiguousarray(Wv[hsl].T),
                "bq": np.ascontiguousarray(bq[hsl][:, None]),
                "bk": np.ascontiguousarray(bk[hsl][:, None]),
                "bv": np.ascontiguousarray(bv[hsl][:, None]),
                "qpos": np.ascontiguousarray(q_positions[b]),
                "kpos": np.ascontiguousarray(kv_positions[b]),
                "spermT": sperm,
                "invfreq": invf,
            }
        )

    if "nc" not in _CACHED:
        _CACHED["nc"] = build_bass()
    nc = _CACHED["nc"]

    res = run_bass_kernel_spmd(nc, in_maps, core_ids=list(range(N_CORES)))
    out = np.empty((B, T, C), np.float32)
    for core in range(N_CORES):
        b, hg = divmod(core, N_CORES // B)
        out[b, :, hg * CO : (hg + 1) * CO] = res.results[core]["out"].T
    return out


# revision 23
# speedup vs baseline: 1.0997x; 1.0997x over previous
"""
Bass/Trainium2 kernel for nn_Attention_72456098284196.

Attention module: QKV projections + partial rotary (first 32 of 64 head
channels, all heads) + softmax attention.  B=2, T=2048, C=1024, H=16, D=64.

Sharding: 8 NeuronCores = 2 batches x 4 head-groups (4 heads each).
Pure tensor/batch parallel -> no collectives; host slices inputs and
concatenates outputs.

Device-side math (per core, all matmuls bf16 with fp32 PSUM accum):
  qT[c,t] = WqT.T @ xqT   (c = 4 heads x 64 ch, two 128-partition c-tiles)
  rotary:  rq = q * cos + (S @ q) * sin, with S the signed splice
           permutation (host constant) and cos/sin built on device from
           the positions input (range-reduced Sin on ScalarE).
  scores^T[s,t] = rkT.T @ rqT per head (K=64, two heads row-packed into
           the 64x128 PE tiling mode)
  expT = Exp(scores^T * 1/sqrt(64)) on ScalarE -> bf16
  outT[d,t] accum over s of [v | 1].T @ expT  (M=65: row 64 accumulates
           the softmax denominator for free)
  out = outT[0:64] * (1/outT[64]) broadcast -> DMA out.
"""

import math
import sys

import numpy as np

if "/opt/trn_rl_repo" not in sys.path:
    sys.path.insert(0, "/opt/trn_rl_repo")

import concourse.bass as bass  # noqa: E402
import concourse.mybir as mybir  # noqa: E402
import concourse.tile as tile  # noqa: E402
from concourse import bacc  # noqa: E402
from concourse.bass_utils import run_bass_kernel_spmd  # noqa: E402

B, T, C = 2, 2048, 1024
NUM_HEADS = 16
HEAD_DIM = 64
N_CORES = 8
HEADS_PER_CORE = NUM_HEADS // (N_CORES // B)  # 4
CO = HEADS_PER_CORE * HEAD_DIM  # 256 out channels per core
N_ROT = 32  # rotated channels per head
MAX_WAVELENGTH = 8192.0

F32 = mybir.dt.float32
BF16 = mybir.dt.bfloat16
I32 = mybir.dt.int32

P = 128  # partitions
TCH = 512  # t chunk (PSUM bank)
KCH = C // P  # 8 contraction chunks
NCT = CO // P  # 2 c-tiles (each = 2 heads x 64)
NST = T // P  # 16 s tiles
NTC = T // TCH  # 4 t chunks
SCALE = 1.0 / math.sqrt(HEAD_DIM)
TWO_PI = 2.0 * math.pi


def _inv_freq_col() -> np.ndarray:
    """[128,1] f32 per-partition inverse frequency for a 2-head c-tile.

    Per 64-ch head slot: d<32 -> rotary inv freq (pairs repeated),
    d>=32 -> 0 (cos(0)=1, sin(0)=0 => passthrough channels survive the
    uniform rotary combine unchanged).
    """
    num_bands = N_ROT // 2  # 16
    freq = MAX_WAVELENGTH ** (
        2.0 / N_ROT * np.linspace(0.0, num_bands, num_bands, dtype=np.float64)
    )
    inv = np.repeat(1.0 / freq, 2)  # [32]
    col = np.zeros((P, 1), np.float32)
    for o in (0, 64):
        col[o : o + N_ROT, 0] = inv
    return col


def _splice_matrix_T() -> np.ndarray:
    """lhsT for the splice matmul, [128,128] bf16 (exact in bf16).

    S maps q -> splice(q) per 2-head c-tile: for rotated pairs
    (2i, 2i+1): out[2i] = -q[2i+1], out[2i+1] = q[2i]; passthrough
    channels -> 0.  Returns S.T so that matmul(out, S.T, q) = S @ q.
    """
    S = np.zeros((P, P), np.float32)
    for o in (0, 64):
        for i in range(N_ROT // 2):
            S[o + 2 * i, o + 2 * i + 1] = -1.0
            S[o + 2 * i + 1, o + 2 * i] = 1.0
    return np.ascontiguousarray(S.T).astype(mybir.dt.np(BF16))


def build_bass() -> bass.Bass:
    nc = bacc.Bacc()

    xq_ext = nc.declare_dram_parameter("xqT", [C, T], F32, isOutput=False)
    xkv_ext = nc.declare_dram_parameter("xkvT", [C, T], F32, isOutput=False)
    wq_ext = nc.declare_dram_parameter("wqT", [C, CO], F32, isOutput=False)
    wk_ext = nc.declare_dram_parameter("wkT", [C, CO], F32, isOutput=False)
    wv_ext = nc.declare_dram_parameter("wvT", [C, CO], F32, isOutput=False)
    bq_ext = nc.declare_dram_parameter("bq", [CO, 1], F32, isOutput=False)
    bk_ext = nc.declare_dram_parameter("bk", [CO, 1], F32, isOutput=False)
    bv_ext = nc.declare_dram_parameter("bv", [CO, 1], F32, isOutput=False)
    qpos_ext = nc.declare_dram_parameter("qpos", [T], I32, isOutput=False)
    kpos_ext = nc.declare_dram_parameter("kpos", [T], I32, isOutput=False)
    sperm_ext = nc.declare_dram_parameter("spermT", [P, P], BF16, isOutput=False)
    invf_ext = nc.declare_dram_parameter("invfreq", [P, 1], F32, isOutput=False)
    out_ext = nc.declare_dram_parameter("out", [CO, T], F32, isOutput=True)

    ExpF = mybir.ActivationFunctionType.Exp
    SinF = mybir.ActivationFunctionType.Sin
    Mul = mybir.AluOpType.mult
    Add = mybir.AluOpType.add

    with tile.TileContext(nc) as tc:
        with (
            tc.tile_pool(name="consts", bufs=1) as consts,
            tc.tile_pool(name="persist", bufs=1) as persist,
        ):
            # ---------------- constants ----------------
            sperm_sb = consts.tile([P, P], BF16, tag="sperm")
            nc.sync.dma_start(out=sperm_sb[:], in_=sperm_ext[:, :])
            invf_sb = consts.tile([P, 1], F32, tag="invf")
            nc.sync.dma_start(out=invf_sb[:], in_=invf_ext[:, :])
            bias_cols = {}
            for name, ext in (("q", bq_ext), ("k", bk_ext)):
                for ct in range(NCT):
                    t_ = consts.tile([P, 1], F32, tag=f"b{name}{ct}")
                    nc.sync.dma_start(out=t_[:], in_=ext[ct * P : (ct + 1) * P, :])
                    bias_cols[(name, ct)] = t_
            bvb_sb = consts.tile([P, CO], F32, tag="bvb")
            nc.sync.dma_start(
                out=bvb_sb[:],
                in_=bass.AP(tensor=bv_ext, offset=0, ap=[[0, P], [1, CO]]),
            )
            halfpi = consts.tile([P, 1], F32, tag="halfpi")
            nc.vector.memset(halfpi[:], math.pi / 2.0)

            rot_sb = {}  # ("q"/"k", ct) -> [128, T] bf16 rotated q/k heads
            for name in ("q", "k"):
                for ct in range(NCT):
                    rot_sb[(name, ct)] = persist.tile(
                        [P, T], BF16, tag=f"r{name}{ct}", name=f"r{name}{ct}"
                    )
            v_sb = [
                persist.tile([P, HEADS_PER_CORE, HEAD_DIM + 1], BF16,
                             tag=f"v{st}", name=f"v{st}")
                for st in range(NST)
            ]

            with tc.tile_pool(name="tables", bufs=1) as tables:
                from contextlib import ExitStack

                proj_stack = ExitStack()
                loads = proj_stack.enter_context(
                    tc.tile_pool(name="loads", bufs=2)
                )
                xw = proj_stack.enter_context(tc.tile_pool(name="xw", bufs=1))
                projtmp = proj_stack.enter_context(
                    tc.tile_pool(name="projtmp", bufs=2)
                )
                psproj = proj_stack.enter_context(
                    tc.tile_pool(name="psproj", bufs=1, space="PSUM")
                )
                psproj2 = proj_stack.enter_context(
                    tc.tile_pool(name="psproj2", bufs=2, space="PSUM")
                )
                rots = proj_stack.enter_context(
                    tc.tile_pool(name="rotscratch", bufs=1)
                )

                # ------------- load + cast weights and x -------------
                # xq + wq first so q-projections start immediately; the
                # k-outer projection loops below consume chunks as they
                # land.  xkv casts split DVE/GpSimd.
                w_bf, x_bf = {}, {}

                def load_w(name, ext, k):
                    wf = loads.tile([P, CO], F32, tag="wf32",
                                    name=f"wf{name}{k}")
                    nc.sync.dma_start(out=wf[:], in_=ext[k * P : (k + 1) * P, :])
                    wb = xw.tile([P, CO], BF16, tag=f"w{name}{k}",
                                 name=f"w{name}{k}")
                    nc.vector.tensor_copy(wb[:], wf[:])
                    w_bf[(name, k)] = wb

                def load_x(name, ext, k, gp):
                    xf = loads.tile([P, T], F32, tag="xf32",
                                    name=f"xf{name}{k}")
                    # split each chunk in half across two DMA trigger
                    # engines for queue parallelism
                    eng = [nc.sync, nc.scalar, nc.gpsimd][k % 3]
                    eng2 = [nc.scalar, nc.gpsimd, nc.sync][k % 3]
                    eng.dma_start(
                        out=xf[0:64, :], in_=ext[k * P : k * P + 64, :]
                    )
                    eng2.dma_start(
                        out=xf[64:P, :], in_=ext[k * P + 64 : (k + 1) * P, :]
                    )
                    xb = xw.tile([P, T], BF16, tag=f"x{name}{k}",
                                 name=f"x{name}{k}")
                    (nc.gpsimd if gp else nc.vector).tensor_copy(xb[:], xf[:])
                    x_bf[(name, k)] = xb

                for k in range(KCH):
                    load_x("q", xq_ext, k, gp=(k in (3, 6)))
                    load_w("q", wq_ext, k)
                for k in range(KCH):
                    load_x("kv", xkv_ext, k, gp=(k in (3, 6)))
                    load_w("k", wk_ext, k)
                    load_w("v", wv_ext, k)

                # ------------- rotary cos/sin tables (q first; the k
                # tables reuse the same tiles after the q projections) ----
                cos_t = tables.tile([P, T], F32, tag="cos", name="cos_t")
                sin_t = tables.tile([P, T], F32, tag="sin", name="sin_t")

                def build_tables(ext, tbl_name):
                    pos_i = rots.tile([P, T], I32, tag="rs_a",
                                      name=f"posi{tbl_name}")
                    nc.sync.dma_start(
                        out=pos_i[:],
                        in_=bass.AP(tensor=ext, offset=0, ap=[[0, P], [1, T]]),
                    )
                    pos_f = rots.tile([P, T], F32, tag="rs_b",
                                      name=f"posf{tbl_name}")
                    nc.vector.tensor_copy(pos_f[:], pos_i[:])
                    rad = rots.tile([P, T], F32, tag="rs_c",
                                    name=f"rad{tbl_name}")
                    nc.vector.tensor_scalar_mul(rad[:], pos_f[:], invf_sb[:])
                    # range reduction: x_red = rad - rnd(rad/2pi)*2pi
                    # (the f32->i32 cast rounds to nearest)
                    n_i = rots.tile([P, T], I32, tag="rs_a",
                                    name=f"ni{tbl_name}")
                    nc.vector.tensor_scalar_mul(n_i[:], rad[:], 1.0 / TWO_PI)
                    n_f = rots.tile([P, T], F32, tag="rs_b",
                                    name=f"nf{tbl_name}")
                    nc.vector.tensor_copy(n_f[:], n_i[:])
                    xred = rots.tile([P, T], F32, tag="rs_d",
                                     name=f"xred{tbl_name}")
                    nc.vector.scalar_tensor_tensor(
                        xred[:], n_f[:], -TWO_PI, rad[:], op0=Mul, op1=Add
                    )
                    # Sin LUT is only valid within ~[-pi, pi]:
                    # sin = Sin(xred); cos = Sin(pi/2 - |xred|)
                    axred = rots.tile([P, T], F32, tag="rs_c",
                                      name=f"axred{tbl_name}")
                    nc.scalar.activation(
                        axred[:], xred[:], mybir.ActivationFunctionType.Abs
                    )
                    nc.scalar.activation(sin_t[:], xred[:], SinF)
                    nc.scalar.activation(
                        cos_t[:], axred[:], SinF, bias=halfpi[:], scale=-1.0
                    )

                build_tables(qpos_ext, "q")

                # ------------- q/k projections + rotary -------------
                # k-outer: the 4 t-chunk PSUMs of one c-tile accumulate as
                # x chunks land, so the PE is paced by DMA, not stalled.
                def project_rotate(name, xsrc, ct):
                    dst = rot_sb[(name, ct)]
                    qsb = projtmp.tile([P, T], BF16, tag="qlin",
                                       name=f"qlin{name}{ct}")
                    pss = [
                        psproj.tile([P, TCH], F32, tag=f"pslin{tci}",
                                    name=f"ps{name}{ct}_{tci}")
                        for tci in range(NTC)
                    ]
                    for k in range(KCH):
                        for tci in range(NTC):
                            tsl = slice(tci * TCH, (tci + 1) * TCH)
                            nc.tensor.matmul(
                                pss[tci][:],
                                w_bf[(name, k)][:, ct * P : (ct + 1) * P],
                                x_bf[(xsrc, k)][:, tsl],
                                start=(k == 0),
                                stop=(k == KCH - 1),
                            )
                    for tci in range(NTC):
                        tsl = slice(tci * TCH, (tci + 1) * TCH)
                        # q = Wx + b  (copy+bias+cast, PSUM -> bf16 SBUF)
                        nc.vector.tensor_scalar_add(
                            qsb[:, tsl], pss[tci][:], bias_cols[(name, ct)][:]
                        )
                        # splice(q) via PE
                        ps2 = psproj2.tile([P, TCH], F32, tag="psspl",
                                          name=f"ps2{name}{ct}_{tci}")
                        nc.tensor.matmul(
                            ps2[:], sperm_sb[:], qsb[:, tsl],
                            start=True, stop=True,
                        )
                        # rq = q*cos + splice(q)*sin
                        t1 = projtmp.tile([P, TCH], F32, tag="rot1",
                                          name=f"t1{name}{ct}_{tci}")
                        nc.vector.tensor_mul(t1[:], qsb[:, tsl], cos_t[:, tsl])
                        t2 = projtmp.tile([P, TCH], F32, tag="rot2",
                                          name=f"t2{name}{ct}_{tci}")
                        nc.vector.tensor_mul(t2[:], ps2[:], sin_t[:, tsl])
                        nc.vector.tensor_add(dst[:, tsl], t1[:], t2[:])

                project_rotate("q", "q", 0)
                project_rotate("q", "q", 1)
                build_tables(kpos_ext, "k")
                project_rotate("k", "kv", 0)
                project_rotate("k", "kv", 1)

                # ------------- v projection ([s, c] layout) -------------
                for st in range(NST):
                    vt = v_sb[st]
                    psv = psproj2.tile([P, CO], F32, tag="psv", name=f"psv{st}")
                    for k in range(KCH):
                        nc.tensor.matmul(
                            psv[:],
                            x_bf[("kv", k)][:, st * P : (st + 1) * P],
                            w_bf[("v", k)][:],
                            start=(k == 0),
                            stop=(k == KCH - 1),
                        )
                    nc.vector.tensor_add(
                        vt[:, :, 0:HEAD_DIM],
                        psv[:].rearrange("p (h d) -> p h d", h=HEADS_PER_CORE),
                        bvb_sb[:].rearrange("p (h d) -> p h d", h=HEADS_PER_CORE),
                    )
                    nc.vector.memset(vt[:, :, HEAD_DIM : HEAD_DIM + 1], 1.0)

                proj_stack.close()

            # ---------------- attention ----------------
            # Per (pair, t-half): block 1 = row-packed K=64 scores (64x128
            # PE tiling mode) + exp to SBUF; block 2 = V matmuls (full
            # 128-row mode).  Keeping the two PE tiling modes in separate
            # blocks avoids a mode-switch drain per s-tile, so the paired
            # score matmuls actually run concurrently; the V block of one
            # iteration overlaps the exp of the next on ScalarE.
            TH = 1024  # t half width
            with (
                tc.tile_pool(name="pssc", bufs=2, space="PSUM") as pssc,
                tc.tile_pool(name="psvacc", bufs=2, space="PSUM") as psvacc,
                tc.tile_pool(name="expp", bufs=48) as expp,
                tc.tile_pool(name="outp", bufs=2) as outp,
                tc.tile_pool(name="small", bufs=2) as smallp,
            ):
                for it, (pair, th) in enumerate(
                    [(p_, t_) for p_ in range(NCT) for t_ in range(2)]
                ):
                    rk = rot_sb[("k", pair)]
                    rq = rot_sb[("q", pair)]
                    # -------- block 1: scores + exp --------
                    etiles = []
                    for st in range(NST):
                        ssl = slice(st * P, (st + 1) * P)
                        psA = pssc.tile([P, TH], F32, tag="scps",
                                        name=f"scA{it}_{st}")
                        psB = pssc.tile([P, TH], F32, tag="scps",
                                        name=f"scB{it}_{st}")
                        for rows, pst in ((slice(0, 64), psA),
                                          (slice(64, P), psB)):
                            tp = (rows.start, 0)
                            for tcc in range(2):
                                tsl = slice(th * TH + tcc * TCH,
                                            th * TH + (tcc + 1) * TCH)
                                psl = slice(tcc * TCH, (tcc + 1) * TCH)
                                nc.tensor.matmul(
                                    pst[:, psl], rk[rows, ssl],
                                    rq[rows, tsl],
                                    start=True, stop=True,
                                    tile_position=tp,
                                )
                        eA = expp.tile([P, TH], BF16, tag="exp",
                                       name=f"eA{it}_{st}")
                        eB = expp.tile([P, TH], BF16, tag="exp",
                                       name=f"eB{it}_{st}")
                        nc.scalar.activation(eA[:], psA[:], ExpF, scale=SCALE)
                        nc.scalar.activation(eB[:], psB[:], ExpF, scale=SCALE)
                        etiles.append((eA, eB))
                    # -------- block 2: V accumulation --------
                    vps = [
                        psvacc.tile([HEAD_DIM + 1, TH], F32, tag="vacc",
                                    name=f"vacc{it}_{s}")
                        for s in range(2)
                    ]
                    for st in range(NST):
                        for sub in range(2):
                            h = pair * 2 + sub
                            e = etiles[st][sub]
                            for tcc in range(2):
                                psl = slice(tcc * TCH, (tcc + 1) * TCH)
                                nc.tensor.matmul(
                                    vps[sub][:, psl],
                                    v_sb[st][:, h, :],
                                    e[:, psl],
                                    start=(st == 0),
                                    stop=(st == NST - 1),
                                )
                    # -------- epilogue --------
                    vcp = [
                        outp.tile([HEAD_DIM + 1, TH], F32, tag="vcp",
                                  name=f"vcp{it}_{s}")
                        for s in range(2)
                    ]
                    for sub in range(2):
                        nc.vector.tensor_copy(vcp[sub][:], vps[sub][:])
                    dn = smallp.tile([4, TCH], F32, tag="dn", name=f"dn{it}")
                    for sub in range(2):
                        nc.sync.dma_start(
                            out=dn[2 * sub : 2 * sub + 2, :],
                            in_=vcp[sub][HEAD_DIM : HEAD_DIM + 1, :],
                        )
                    rc4 = smallp.tile([4, TCH], F32, tag="rc4", name=f"rc4{it}")
                    nc.vector.reciprocal(rc4[:], dn[:])
                    for sub in range(2):
                        h = pair * 2 + sub
                        rcb = smallp.tile([HEAD_DIM, TH], F32, tag="rcb",
                                          name=f"rcb{it}_{sub}")
                        for j in range(2):
                            r1 = smallp.tile([1, TCH], F32, tag="r1",
                                             name=f"r1_{it}_{sub}{j}")
                            nc.sync.dma_start(
                                out=r1[:],
                                in_=rc4[2 * sub + j : 2 * sub + j + 1, :],
                            )
                            nc.gpsimd.partition_broadcast(
                                rcb[:, j * TCH : (j + 1) * TCH], r1[:],
                                channels=HEAD_DIM,
                            )
                        osb = outp.tile([HEAD_DIM, TH], F32, tag="osb",
                                        name=f"osb{it}_{sub}")
                        nc.vector.tensor_mul(
                            osb[:], vcp[sub][0:HEAD_DIM, :], rcb[:]
                        )
                        nc.sync.dma_start(
                            out=out_ext[h * HEAD_DIM : (h + 1) * HEAD_DIM,
                                        th * TH : (th + 1) * TH],
                            in_=osb[:],
                        )
    nc.finalize()
    return nc


_CACHED = {}


def kernel(x_q, x_kv, q_positions, kv_positions, Wq, bq, Wk, bk, Wv, bv):
    x_q = np.asarray(x_q, np.float32)
    x_kv = np.asarray(x_kv, np.float32)
    q_positions = np.asarray(q_positions, np.int32)
    kv_positions = np.asarray(kv_positions, np.int32)
    Wq, Wk, Wv = (np.asarray(w, np.float32) for w in (Wq, Wk, Wv))
    bq, bk, bv = (np.asarray(b, np.float32) for b in (bq, bk, bv))

    sperm = _splice_matrix_T()
    invf = _inv_freq_col()

    in_maps = []
    for core in range(N_CORES):
        b, hg = divmod(core, N_CORES // B)
        hsl = slice(hg * CO, (hg + 1) * CO)
        in_maps.append(
            {
                "xqT": np.ascontiguousarray(x_q[b].T),
                "xkvT": np.ascontiguousarray(x_kv[b].T),
                "wqT": np.ascontiguousarray(Wq[hsl].T),
                "wkT": np.ascontiguousarray(Wk[hsl].T),
                "wvT": np.ascontiguousarray(Wv[hsl].T),
                "bq": np.ascontiguousarray(bq[hsl][:, None]),
                "bk": np.ascontiguousarray(bk[hsl][:, None]),
                "bv": np.ascontiguousarray(bv[hsl][:, None]),
                "qpos": np.ascontiguousarray(q_positions[b]),
                "kpos": np.ascontiguousarray(kv_positions[b]),
                "spermT": sperm,
                "invfreq": invf,
            }
        )

    if "nc" not in _CACHED:
        _CACHED["nc"] = build_bass()
    nc = _CACHED["nc"]

    res = run_bass_kernel_spmd(nc, in_maps, core_ids=list(range(N_CORES)))
    out = np.empty((B, T, C), np.float32)
    for core in range(N_CORES):
        b, hg = divmod(core, N_CORES // B)
        out[b, :, hg * CO : (hg + 1) * CO] = res.results[core]["out"].T
    return out
